# revision 39
# baseline (speedup 1.0000x reference)
"""NormLinearAttention Trainium2 kernel — 8-core sequence-parallel Bass/Tile.

Math (reference):
    q = k = elu(heads(x @ Wqk + bqk));  v = heads(silu(x @ Wv + bv))
    u = silu(x @ Wu + bu)
    kv[b,h] = k^T v  (contract over sequence);  att = q @ kv
    y = (u * layernorm(att)) @ Wo + bo

Sharding: each of 8 cores owns 512 tokens of each batch (2048 tokens total).
Per-core partial kv is AllReduce-summed across cores in 4 chunks (1 MB each,
issued per head-quarter as phase 1 produces them, overlapped with the
u-projection); everything else is local.

v2 layout/schedule notes (changes vs v1):
  - q is spilled to DRAM token-major and re-loaded in phase 3 with the XBAR
    transposing DMA (dma_start_transpose), killing 256 PE transposes plus
    their PSUM/DVE copies.
  - v's silu is computed as 0.5*x*(1+tanh(x/2)) so all of phase 1 uses the
    exp activation-table set (elu needs Exp); avoids ~60 ACT_TABLE_LOADs.
  - rstd = exp(-0.5*ln(var+eps)) (natural_log_exp set; Rsqrt is banned and
    Sqrt+reciprocal costs a serial 3.3us DVE reciprocal per batch).
  - LN scalars are partition-broadcast on GpSimd (SBUF->SBUF), not via PE
    ones-matmuls into PSUM: frees 2 PSUM banks and avoids PE-FIFO stalls.
  - kv AllReduce chunked by head-quarter; phase 3 att starts as chunks land.
  - u/w weight loads + uT spill + y stores ride the scalar DMA queue so the
    sync queue keeps phase-critical traffic only; phase-1 issue order puts
    wq/wv ahead of the 8 MB xT load so the PE starts ~8us in, not 61us.
  - y is written bf16 (host upcasts); halves the output-DMA tail.
All matmuls bf16 (fp32 PSUM accumulate); elementwise/LN math fp32.
"""

import sys

if "/opt/trn_rl_repo" not in sys.path:
    sys.path.insert(0, "/opt/trn_rl_repo")

import numpy as np
import ml_dtypes

B, N, E = 4, 4096, 2048
H_DIM, HEADS, DH = 2048, 16, 128
N_CORES = 8
NL = N // N_CORES          # 512 tokens per (core, batch)
TL = B * NL                # 2048 local tokens per core
ET = E // 128              # 16 contraction tiles
JT = H_DIM // 128          # 16 hidden tiles
NJQ = 4                    # process hidden dim in 4 quarters of 512
TOK_B = NL // 128          # 4 token tiles per batch
LN_EPS = 1e-5

_BUILT = {}


def _build(flags, debug=False):
    import concourse.bacc as bacc
    import concourse.mybir as mybir
    import concourse.tile as tile

    has_bqv, has_bu, has_bo, has_affine = flags
    f32 = mybir.dt.float32
    bf16 = mybir.dt.bfloat16

    nc = bacc.Bacc("TRN2", target_bir_lowering=False, debug=False,
                   num_devices=N_CORES)

    t = {}
    t["xT"] = nc.dram_tensor("xT", [E, TL], bf16, kind="ExternalInput").ap()
    # weights arrive pre-rearranged from the host so every device load is
    # contiguous (strided 1KB-chunk DMAs run at ~34 GB/s vs 358 contiguous)
    t["wqk"] = nc.dram_tensor("wqk", [NJQ, 128, ET, 512], bf16,
                              kind="ExternalInput").ap()
    t["wv"] = nc.dram_tensor("wv", [NJQ, 128, ET, 512], bf16,
                             kind="ExternalInput").ap()
    t["wu"] = nc.dram_tensor("wu", [NJQ, 128, ET, 512], bf16,
                             kind="ExternalInput").ap()
    t["wo"] = nc.dram_tensor("wo", [128, JT, E], bf16,
                             kind="ExternalInput").ap()
    if has_bqv:
        t["bqk_r"] = nc.dram_tensor("bqk_r", [1, H_DIM], bf16,
                                    kind="ExternalInput").ap()
        t["bv_r"] = nc.dram_tensor("bv_r", [1, H_DIM], bf16,
                                   kind="ExternalInput").ap()
    if has_bo:
        t["bo_r"] = nc.dram_tensor("bo_r", [1, E], bf16,
                                   kind="ExternalInput").ap()
    if has_bu:
        t["bu_c"] = nc.dram_tensor("bu_c", [128, JT], f32,
                                   kind="ExternalInput").ap()
    if has_affine:
        t["g_c"] = nc.dram_tensor("g_c", [128, JT], f32,
                                  kind="ExternalInput").ap()
        t["b_c"] = nc.dram_tensor("b_c", [128, JT], f32,
                                  kind="ExternalInput").ap()
    t["y"] = nc.dram_tensor("y", [TL, E], bf16, kind="ExternalOutput").ap()

    dbg = None
    if debug:
        dbg = {
            "q": nc.dram_tensor("dbg_q", [TL, H_DIM], bf16,
                                kind="ExternalOutput").ap(),
            "uT": nc.dram_tensor("dbg_uT", [B * 128, JT * NL], bf16,
                                 kind="ExternalOutput").ap(),
            "kvout": nc.dram_tensor("dbg_kvout", [NJQ * B * 128, 4 * DH],
                                    f32, kind="ExternalOutput").ap(),
            "att": nc.dram_tensor("dbg_att", [B * 128, HEADS * NL], bf16,
                                  kind="ExternalOutput").ap(),
            "zT": nc.dram_tensor("dbg_zT", [B * 128, JT * NL], bf16,
                                 kind="ExternalOutput").ap(),
        }
    with tile.TileContext(nc) as tc:
        _body(nc, tc, tile, mybir, f32, bf16, t, flags, dbg)
    nc.compile()
    return nc


def _body(nc, tc, tile, mybir, f32, bf16, t, flags, dbg=None):
    Act = mybir.ActivationFunctionType
    Alu = mybir.AluOpType
    has_bqv, has_bu, has_bo, has_affine = flags

    with (
        tc.tile_pool(name="consts", bufs=1) as consts,
        tc.tile_pool(name="p3small", bufs=1) as p3small,
        tc.tile_pool(name="dram", bufs=1, space="DRAM") as dram,
    ):
        ones_col = consts.tile([128, 1], bf16)
        nc.vector.memset(ones_col, 1.0)
        eps_sb = consts.tile([1, 1], f32)
        nc.vector.memset(eps_sb, LN_EPS)
        utoken = consts.tile([1, 1], f32)
        nc.vector.memset(utoken, 0.0)
        if has_bqv or has_bo:
            ones_bf = consts.tile([1, 128], bf16)
            nc.vector.memset(ones_bf, 1.0)
        if has_bqv:
            bqk_sb = consts.tile([1, H_DIM], bf16)
            nc.sync.dma_start(bqk_sb[:], t["bqk_r"][:])
            bv_sb = consts.tile([1, H_DIM], bf16)
            nc.sync.dma_start(bv_sb[:], t["bv_r"][:])
        if has_bo:
            bo_sb = consts.tile([1, E], bf16)
            nc.sync.dma_start(bo_sb[:], t["bo_r"][:])
        if has_bu:
            bu_sb = consts.tile([128, JT], f32)
            nc.sync.dma_start(bu_sb[:], t["bu_c"][:])
        if has_affine:
            g_sb = consts.tile([128, JT], f32)
            nc.sync.dma_start(g_sb[:], t["g_c"][:])
            b_sb = consts.tile([128, JT], f32)
            nc.sync.dma_start(b_sb[:], t["b_c"][:])

        q_dram = dram.tile([TL, H_DIM], bf16)     # token-major q spill
        uT_dram = dram.tile([128, B, JT, NL], bf16)
        # kv collective chunks: one per head-quarter, laid out
        # [B, 128, 4, DH] so the kv stores/loads are direct 2KB-run DMAs
        cc_in = []
        cc_out = []
        for jq in range(NJQ):
            cc_in_t = dram.tile([B, 128, 4, DH], f32, name=f"cc_in{jq}")
            cc_in.append(cc_in_t)
            cc_out_t = dram.tile([B, 128, 4, DH], f32,
                                 addr_space="Shared", name=f"cc_out{jq}")
            cc_out.append(cc_out_t)

        # small early pools: their SBUF is disjoint from xt/w1, so the
        # phase-3 qb/kv loads can prefetch during phase 2 (no WAR on xt)
        kvf_ctx = tc.tile_pool(name="kvf", bufs=1)
        kvf = kvf_ctx.__enter__()
        qb_ctx = tc.tile_pool(name="qbp", bufs=1)
        qbp = qb_ctx.__enter__()

        with tc.tile_pool(name="xt_pool", bufs=1) as xt_pool:
            xt = xt_pool.tile([128, ET, TL], bf16)   # 8 MB resident ph1-2

            # w2 spans phases 1+2 with SBUF addresses disjoint from w1/st1,
            # so the wu prefetch overlaps phase 1 (no phase-boundary stall)
            w2_ctx = tc.tile_pool(name="w2", bufs=1)
            w2 = w2_ctx.__enter__()

            # ---------------- phase 1: q/v projections + partial kv --------
            with (
                tc.tile_pool(name="w1", bufs=1) as w1,
                tc.tile_pool(name="st1", bufs=1) as st1,
                tc.tile_pool(name="ps_proj", bufs=1, space="PSUM") as psp,
                tc.tile_pool(name="ps_kv", bufs=1, space="PSUM") as pskv,
            ):
                # phase-1 weights first in the DMA queue so the PE can start
                # ~8us in instead of waiting behind the 8 MB xT load
                wq_sb = {}
                wv_sb = {}
                wq_sb[0] = w1.tile([128, ET, 512], bf16, tag="wq", bufs=2,
                                   name="wq0")
                nc.sync.dma_start(wq_sb[0][:], t["wqk"][0])
                wv_sb[0] = w1.tile([128, ET, 512], bf16, tag="wv", bufs=2,
                                   name="wv0")
                nc.scalar.dma_start(wv_sb[0][:], t["wv"][0])

                for tt in range(ET):
                    nc.sync.dma_start(
                        xt[:, tt],
                        t["xT"].rearrange("(t p) n -> t p n", p=128)[tt])

                wu_pre = []
                for jqu in range(2):
                    wu_q = w2.tile([128, ET, 512], bf16, tag="wu", bufs=2)
                    nc.scalar.dma_start(wu_q[:], t["wu"][jqu])
                    wu_pre.append(wu_q)

                for jq in range(NJQ):
                    if jq > 0:
                        wq_sb[jq] = w1.tile([128, ET, 512], bf16, tag="wq",
                                            bufs=2, name=f"wq{jq}")
                        nc.sync.dma_start(wq_sb[jq][:], t["wqk"][jq])
                        wv_sb[jq] = w1.tile([128, ET, 512], bf16, tag="wv",
                                            bufs=2, name=f"wv{jq}")
                        nc.sync.dma_start(wv_sb[jq][:], t["wv"][jq])
                    for b in range(B):
                        q_tiles, v_tiles = [], []
                        for tk in range(TOK_B):
                            tok0 = b * NL + tk * 128
                            q_ps = psp.tile([128, 512], f32, tag="qps", bufs=2)
                            v_ps = psp.tile([128, 512], f32, tag="vps", bufs=2)
                            for tt in range(ET):
                                lhs = xt[:, tt, tok0:tok0 + 128]
                                nc.tensor.matmul(q_ps[:], lhs,
                                                 wq_sb[jq][:, tt],
                                                 start=(tt == 0), stop=False)
                                nc.tensor.matmul(
                                    v_ps[:], lhs, wv_sb[jq][:, tt],
                                    start=(tt == 0),
                                    stop=(not has_bqv and tt == ET - 1))
                            if has_bqv:
                                nc.tensor.matmul(
                                    q_ps[:], ones_bf[:],
                                    bqk_sb[:, jq * 512:(jq + 1) * 512],
                                    start=False, stop=True)
                                nc.tensor.matmul(
                                    v_ps[:], ones_bf[:],
                                    bv_sb[:, jq * 512:(jq + 1) * 512],
                                    start=False, stop=True)

                            # elu(q) = (max(q,0) - 1) + exp(min(q, 0))
                            tmin = st1.tile([128, 512], f32, tag="tmin",
                                            bufs=2)
                            nc.vector.tensor_scalar_min(tmin[:], q_ps[:], 0.0)
                            texp = st1.tile([128, 512], f32, tag="texp",
                                            bufs=1)
                            nc.scalar.activation(texp[:], tmin[:], Act.Exp)
                            trelu = st1.tile([128, 512], f32, tag="trelu",
                                             bufs=1)
                            nc.vector.tensor_scalar(trelu[:], q_ps[:], 0.0,
                                                    -1.0, Alu.max, Alu.add)
                            q_bf = st1.tile([128, 512], bf16, tag="qbf",
                                            bufs=4)
                            nc.vector.tensor_add(q_bf[:], trelu[:], texp[:])
                            # silu(v) = 0.5*v*(1+tanh(v/2)) — keeps the exp
                            # table set loaded (tanh lives in it; Silu not)
                            vth = st1.tile([128, 512], f32, tag="vth", bufs=1)
                            nc.scalar.activation(vth[:], v_ps[:], Act.Tanh,
                                                 scale=0.5)
                            vsg = st1.tile([128, 512], f32, tag="vsg", bufs=1)
                            nc.vector.tensor_scalar(vsg[:], vth[:], 0.5, 0.5,
                                                    Alu.mult, Alu.add)
                            v_bf = st1.tile([128, 512], bf16, tag="vbf",
                                            bufs=4)
                            nc.vector.tensor_mul(v_bf[:], v_ps[:], vsg[:])
                            q_tiles.append(q_bf)
                            v_tiles.append(v_bf)

                            # spill q token-major; phase 3 reloads it through
                            # the XBAR transposing DMA
                            nc.sync.dma_start(
                                q_dram[tok0:tok0 + 128,
                                       jq * 512:(jq + 1) * 512],
                                q_bf[:])

                        # per-head contiguous kv accumulation: each head owns
                        # a whole PSUM bank (start=True clears the full bank,
                        # so accumulation groups must not share banks)
                        kv_sb = st1.tile([128, 4, DH], f32, tag="kvsb",
                                         bufs=1)
                        for h in range(4):
                            kv_ps = pskv.tile([128, DH], f32, tag="kv",
                                              bufs=2)
                            for tk in range(TOK_B):
                                nc.tensor.matmul(
                                    kv_ps[:],
                                    q_tiles[tk][:, h * 128:(h + 1) * 128],
                                    v_tiles[tk][:, h * 128:(h + 1) * 128],
                                    start=(tk == 0), stop=(tk == TOK_B - 1))
                            nc.vector.tensor_copy(kv_sb[:, h], kv_ps[:])
                        nc.sync.dma_start(cc_in[jq][b], kv_sb[:])

                    # AllReduce this head-quarter as soon as its kv is out;
                    # chunks overlap the phase-1 tail + the u-projection
                    nc.gpsimd.collective_compute(
                        "AllReduce", mybir.AluOpType.add,
                        replica_groups=[list(range(N_CORES))],
                        ins=[cc_in[jq].opt()], outs=[cc_out[jq].opt()])

            # ---------------- phase 2: uT projection (overlaps AR) ---------
            u_st_last = [None]
            with (
                tc.tile_pool(name="u2", bufs=1) as u2,
                tc.tile_pool(name="ps_u", bufs=1, space="PSUM") as psu,
            ):
                for jqu in range(NJQ):
                    if jqu < 2:
                        wu_q = wu_pre[jqu]
                    else:
                        wu_q = w2.tile([128, ET, 512], bf16, tag="wu", bufs=2)
                        nc.scalar.dma_start(wu_q[:], t["wu"][jqu])
                    u_st = u2.tile([128, 4, TL], bf16, tag="ust", bufs=2)
                    for jl in range(4):
                        jt = jqu * 4 + jl
                        ubias = bu_sb[:, jt:jt + 1] if has_bu else 0.0
                        for ch in range(2):
                            u_ps = psu.tile([128, 2, 512], f32, tag="ups",
                                            bufs=2)
                            for tt in range(ET):
                                for c2 in range(2):
                                    c = ch * 2 + c2
                                    nc.tensor.matmul(
                                        u_ps[:, c2],
                                        wu_q[:, tt, jl * 128:(jl + 1) * 128],
                                        xt[:, tt, c * 512:(c + 1) * 512],
                                        start=(tt == 0), stop=(tt == ET - 1))
                            # silu via tanh keeps the exp table set loaded
                            # for the whole kernel (one switch total)
                            uth = u2.tile([128, 2, 512], f32, tag="uth",
                                          bufs=2)
                            nc.scalar.activation(uth[:], u_ps[:], Act.Tanh,
                                                 scale=0.5, bias=ubias)
                            usg = u2.tile([128, 2, 512], f32, tag="usg",
                                          bufs=2)
                            nc.vector.tensor_scalar(usg[:], uth[:], 0.5, 0.5,
                                                    Alu.mult, Alu.add)
                            nc.vector.tensor_mul(
                                u_st[:, jl, ch * 1024:(ch + 1) * 1024]
                                .rearrange("p (a n) -> p a n", a=2),
                                u_ps[:], usg[:])
                    u_st_last[0] = u_st
                    # contiguous spills per (quarter, batch): 4KB runs.
                    # Same ring (sync) as the phase-3 uT loads: ring FIFO
                    # order guarantees load-after-spill without relying on
                    # cross-ring DRAM dependency tracking.
                    for sb in range(B):
                        nc.scalar.dma_start(
                            uT_dram[:, sb, jqu * 4:(jqu + 1) * 4, :],
                            u_st[:, :, sb * NL:(sb + 1) * NL])
                # token carrying "all u activations done" for the chain dep
                nc.vector.tensor_copy(utoken[:], u_st_last[0][0:1, 0, 0:1])
            w2_ctx.__exit__(None, None, None)

        if dbg is not None:
            nc.sync.dma_start(dbg["q"][:], q_dram[:])
            nc.sync.dma_start(
                dbg["uT"].rearrange("(b p) (jt n) -> p b jt n", p=128, jt=JT),
                uT_dram[:])
            for jq in range(NJQ):
                nc.sync.dma_start(
                    dbg["kvout"][jq * B * 128:(jq + 1) * B * 128, :]
                    .rearrange("(b p) (h e) -> b p h e", b=B, h=4),
                    cc_out[jq][:])

        # ------------- phase 3+4: attention, layernorm, output proj --------
        with (
            tc.tile_pool(name="wo_pool", bufs=1) as wo_pool,
            tc.tile_pool(name="st3", bufs=1) as st3,
            tc.tile_pool(name="utp", bufs=1) as utp,
            tc.tile_pool(name="ps_att", bufs=1, space="PSUM") as psa,
            tc.tile_pool(name="ps_sm", bufs=1, space="PSUM") as pssm,
            tc.tile_pool(name="ps_y", bufs=1, space="PSUM") as psy,
        ):
            wo_sb = wo_pool.tile([128, JT, E], bf16)     # 8 MB resident

            def load_wo():
                # emitted late so these triggers (blocked on the xt-space WAR
                # until phase 2 ends) don't park at the head of the sync ring
                # and stall the qb/uT prefetch behind them
                for wq4 in range(4):
                    weng = nc.sync if wq4 % 2 == 0 else nc.gpsimd
                    weng.dma_start(wo_sb[:, wq4 * 4:(wq4 + 1) * 4],
                                   t["wo"][:, wq4 * 4:(wq4 + 1) * 4])

            att_tiles = {}
            sq_tiles = {}
            stat_tiles = {}

            def att_block(b):
                kv_bf = kvf.tile([128, HEADS, DH], bf16, tag="kvbf", bufs=1)
                for jq in range(NJQ):
                    nc.gpsimd.dma_start(
                        kv_bf[:, jq * 4:(jq + 1) * 4], cc_out[jq][b])
                att = st3.tile([128, HEADS, NL], bf16, tag="att", bufs=2)
                att_tiles[b] = att
                sq = st3.tile([128, HEADS, NL], bf16, tag="sq", bufs=1)
                sq_tiles[b] = sq
                for jq in range(NJQ):
                    # one XBAR transpose per head-quarter (not per head):
                    # 16 loads of 512KB pipeline ahead of the att matmuls
                    qb = qbp.tile([128, 4, NL], bf16, tag="qb", bufs=4)
                    nc.sync.dma_start_transpose(
                        qb[:],
                        q_dram[b * NL:(b + 1) * NL,
                               jq * 512:(jq + 1) * 512])
                    for hl in range(4):
                        h = jq * 4 + hl
                        att_ps = psa.tile([128, NL], f32, tag="attps", bufs=3)
                        nc.tensor.matmul(att_ps[:], kv_bf[:, h], qb[:, hl],
                                         start=True, stop=True)
                        nc.vector.tensor_copy(att[:, h], att_ps[:])
                        nc.scalar.activation(sq[:, h], att[:, h], Act.Square)
                if dbg is not None:
                    nc.sync.dma_start(dbg["att"][b * 128:(b + 1) * 128, :],
                                      att[:])

            def stats_block(b):
                att = att_tiles[b]
                sq = sq_tiles.pop(b)
                # LN stats over channels via ones-matmuls (bf16 operands)
                sum_ps = pssm.tile([1, NL], f32, tag="sum", bufs=1)
                ssq_ps = pssm.tile([1, NL], f32, tag="ssq", bufs=1)
                for h in range(HEADS):
                    nc.tensor.matmul(sum_ps[:], ones_col[:], att[:, h],
                                     start=(h == 0), stop=(h == HEADS - 1))
                for h in range(HEADS):
                    nc.tensor.matmul(ssq_ps[:], ones_col[:], sq[:, h],
                                     start=(h == 0), stop=(h == HEADS - 1))
                stat_tiles[b] = (sum_ps, ssq_ps)

            def chain_block(b):
                # small [1,NL] tiles cost full per-partition column space, so
                # the chain recycles three tags by value lifetime
                sum_ps, ssq_ps = stat_tiles.pop(b)
                mean = st3.tile([1, NL], f32, tag="cA", bufs=1, name="mean")
                # bypass-op dep on utoken (written after the last u tile):
                # forces every phase-2 Tanh before the chain's Ln/Exp so the
                # ACT table set switches exactly once
                nc.vector.scalar_tensor_tensor(
                    mean[:], sum_ps[:], 1.0 / H_DIM,
                    utoken.broadcast_to([1, NL]), Alu.mult, Alu.bypass)
                m2 = st3.tile([1, NL], f32, tag="cB", bufs=1, name="m2")
                nc.scalar.activation(m2[:], mean[:], Act.Square)
                var = st3.tile([1, NL], f32, tag="cC", bufs=1, name="var")
                nc.vector.scalar_tensor_tensor(
                    var[:], ssq_ps[:], 1.0 / H_DIM, m2[:],
                    Alu.mult, Alu.subtract)
                lnv = st3.tile([1, NL], f32, tag="cB", bufs=1, name="lnv")
                nc.scalar.activation(lnv[:], var[:], Act.Ln, bias=eps_sb[:])
                rstd = st3.tile([1, NL], f32, tag="cC", bufs=1, name="rstd")
                nc.scalar.activation(rstd[:], lnv[:], Act.Exp, scale=-0.5)
                mr = st3.tile([1, NL], f32, tag="cB", bufs=1, name="mr")
                nc.vector.tensor_mul(mr[:], mean[:], rstd[:])
                rstd_b16 = st3.tile([1, NL], bf16, tag="c16a", bufs=1,
                                    name="rstd_b16")
                nc.vector.tensor_copy(rstd_b16[:], rstd[:])
                mr_b16 = st3.tile([1, NL], bf16, tag="c16b", bufs=1,
                                  name="mr_b16")
                nc.vector.tensor_copy(mr_b16[:], mr[:])
                rstd_bc = st3.tile([128, NL], bf16, tag="rstdbc", bufs=2)
                nc.gpsimd.partition_broadcast(rstd_bc[:], rstd_b16[:])
                mr_bc = st3.tile([128, NL], bf16, tag="mrbc", bufs=2)
                nc.gpsimd.partition_broadcast(mr_bc[:], mr_b16[:])
                return rstd_bc, mr_bc

            ut_loaded = {}

            def load_ut(b):
                uT_b = utp.tile([128, JT, NL], bf16, tag="utb", bufs=1)
                nc.scalar.dma_start(uT_b[:], uT_dram[:, b])
                ut_loaded[b] = uT_b

            zT_tiles = {}

            def zT_block(b, bc):
                rstd_bc, mr_bc = bc
                att = att_tiles.pop(b)
                uT_b = ut_loaded.pop(b)
                zT = st3.tile([128, JT, NL], bf16, tag="zT", bufs=2)
                zT_tiles[b] = zT
                # (att*rstd - mean*rstd) [*g+b] * u — bf16, quarters split
                # across DVE and GpSimd so the zT latency halves
                for e8 in range(8):
                    eng = nc.vector
                    j0 = e8 * 2
                    s1 = st3.tile([128, 2, NL], bf16, tag=f"s1{e8 // 4}",
                                  bufs=1, name="s1")
                    eng.tensor_mul(
                        s1[:], att[:, j0:j0 + 2],
                        rstd_bc[:, None, :].broadcast_to([128, 2, NL]))
                    s2 = st3.tile([128, 2, NL], bf16, tag=f"s2{e8 // 4}",
                                  bufs=1, name="s2")
                    eng.tensor_sub(
                        s2[:], s1[:],
                        mr_bc[:, None, :].broadcast_to([128, 2, NL]))
                    if has_affine:
                        s3 = st3.tile([128, 2, NL], bf16, tag=f"s3{e8 // 4}",
                                      bufs=1, name="s3")
                        for jl in range(2):
                            jt = j0 + jl
                            eng.tensor_scalar(
                                s3[:, jl], s2[:, jl], g_sb[:, jt:jt + 1],
                                b_sb[:, jt:jt + 1], Alu.mult, Alu.add)
                    else:
                        s3 = s2
                    eng.tensor_mul(zT[:, j0:j0 + 2], s3[:],
                                   uT_b[:, j0:j0 + 2])
                if dbg is not None:
                    nc.sync.dma_start(dbg["zT"][b * 128:(b + 1) * 128, :],
                                      zT[:])

            def yproj_block(b):
                zT = zT_tiles.pop(b)
                # y = zT.T @ Wo + bo; e-quarter PSUM tiles (1 bank, bufs=3)
                # so the evacuation copies overlap the next quarter's matmuls
                for tsl in range(TOK_B):
                    ybuf = st3.tile([128, E], bf16, tag="ybuf", bufs=1)
                    for e4 in range(4):
                        e0 = e4 * 512
                        y_ps = psy.tile([128, 512], f32, tag="yps", bufs=3)
                        for ct in range(JT):
                            zslice = zT[:, ct, tsl * 128:(tsl + 1) * 128]
                            nc.tensor.matmul(
                                y_ps[:], zslice,
                                wo_sb[:, ct, e0:e0 + 512],
                                start=(ct == 0),
                                stop=(not has_bo and ct == JT - 1))
                        if has_bo:
                            nc.tensor.matmul(
                                y_ps[:], ones_bf[:],
                                bo_sb[:, e0:e0 + 512],
                                start=False, stop=True)
                        nc.scalar.copy(ybuf[:, e0:e0 + 512], y_ps[:])
                    nc.sync.dma_start(
                        t["y"][b * NL + tsl * 128: b * NL + (tsl + 1) * 128,
                               :],
                        ybuf[:])

            # software-pipelined schedule: PE stream is
            #   att0 att1 stats0 att2 stats1 att3 stats2 yproj0 stats3 ...
            # while each batch's LN chain (ACT/DVE/GpSimd) resolves under the
            # next batches' matmuls.
            # emission order doubles as queue priority: qb/kv/uT loads
            # first (so they prefetch during phase 2), wo quarters late,
            # each batch's LN chain resolving under the next batches' MMs
            att_block(0)
            att_block(1)
            load_ut(0)
            load_ut(1)
            stats_block(0)
            bc0 = chain_block(0)
            zT_block(0, bc0)
            att_block(2)
            stats_block(1)
            bc1 = chain_block(1)
            zT_block(1, bc1)
            att_block(3)
            load_wo()
            load_ut(2)
            stats_block(2)
            bc2 = chain_block(2)
            yproj_block(0)
            zT_block(2, bc2)
            load_ut(3)
            stats_block(3)
            bc3 = chain_block(3)
            yproj_block(1)
            zT_block(3, bc3)
            yproj_block(2)
            yproj_block(3)

        qb_ctx.__exit__(None, None, None)
        kvf_ctx.__exit__(None, None, None)



def _get_nc(flags, debug=False):
    key = (flags, debug)
    if key not in _BUILT:
        _BUILT[key] = _build(flags, debug)
    return _BUILT[key]


def make_in_maps(x, Wqk, bqk, Wv, bv, Wu, bu, Wo, bo, ln_g, ln_b):
    bf16 = ml_dtypes.bfloat16
    f32 = np.float32
    x = np.asarray(x)
    flags = (
        bool(np.any(bqk) or np.any(bv)),
        bool(np.any(bu)),
        bool(np.any(bo)),
        bool(np.any(np.asarray(ln_g) != 1.0) or np.any(ln_b)),
    )
    def _wr(w):  # [E, H_DIM] -> [NJQ, 128, ET, 512] (jq, p, tt, j)
        return np.ascontiguousarray(
            np.asarray(w, f32).astype(bf16)
            .reshape(ET, 128, NJQ, 512).transpose(2, 1, 0, 3))

    shared = {
        "wqk": _wr(Wqk),
        "wv": _wr(Wv),
        "wu": _wr(Wu),
        "wo": np.ascontiguousarray(
            np.asarray(Wo, f32).astype(bf16)
            .reshape(JT, 128, E).transpose(1, 0, 2)),
    }
    if flags[0]:
        shared["bqk_r"] = np.asarray(bqk, f32).astype(bf16).reshape(1, H_DIM)
        shared["bv_r"] = np.asarray(bv, f32).astype(bf16).reshape(1, H_DIM)
    if flags[1]:
        shared["bu_c"] = np.ascontiguousarray(
            np.asarray(bu, f32).reshape(JT, 128).T)
    if flags[2]:
        shared["bo_r"] = np.asarray(bo, f32).astype(bf16).reshape(1, E)
    if flags[3]:
        shared["g_c"] = np.ascontiguousarray(
            np.asarray(ln_g, f32).reshape(JT, 128).T)
        shared["b_c"] = np.ascontiguousarray(
            np.asarray(ln_b, f32).reshape(JT, 128).T)
    in_maps = []
    for c in range(N_CORES):
        xc = np.ascontiguousarray(
            x[:, c * NL:(c + 1) * NL, :].reshape(TL, E).T).astype(bf16)
        in_maps.append({"xT": xc, **shared})
    return flags, in_maps


def kernel(x, Wqk, bqk, Wv, bv, Wu, bu, Wo, bo, ln_g, ln_b, **_unused):
    from concourse.bass_utils import run_bass_kernel_spmd

    flags, in_maps = make_in_maps(x, Wqk, bqk, Wv, bv, Wu, bu, Wo, bo,
                                  ln_g, ln_b)
    nc = _get_nc(flags)
    res = run_bass_kernel_spmd(nc, in_maps, core_ids=list(range(N_CORES)))

    y = np.empty((B, N, E), np.float32)
    for c in range(N_CORES):
        y[:, c * NL:(c + 1) * NL, :] = (
            res.results[c]["y"].astype(np.float32).reshape(B, NL, E))
    return y


# revision 40
# speedup vs baseline: 1.0648x; 1.0648x over previous
"""NormLinearAttention Trainium2 kernel — 8-core sequence-parallel Bass/Tile.

Math (reference):
    q = k = elu(heads(x @ Wqk + bqk));  v = heads(silu(x @ Wv + bv))
    u = silu(x @ Wu + bu)
    kv[b,h] = k^T v  (contract over sequence);  att = q @ kv
    y = (u * layernorm(att)) @ Wo + bo

Sharding: each of 8 cores owns 512 tokens of each batch (2048 tokens total).
Per-core partial kv is AllReduce-summed across cores in 4 chunks (1 MB each,
issued per head-quarter as phase 1 produces them, overlapped with the
u-projection); everything else is local.

v2 layout/schedule notes (changes vs v1):
  - q is spilled to DRAM token-major and re-loaded in phase 3 with the XBAR
    transposing DMA (dma_start_transpose), killing 256 PE transposes plus
    their PSUM/DVE copies.
  - v's silu is computed as 0.5*x*(1+tanh(x/2)) so all of phase 1 uses the
    exp activation-table set (elu needs Exp); avoids ~60 ACT_TABLE_LOADs.
  - rstd = exp(-0.5*ln(var+eps)) (natural_log_exp set; Rsqrt is banned and
    Sqrt+reciprocal costs a serial 3.3us DVE reciprocal per batch).
  - LN scalars are partition-broadcast on GpSimd (SBUF->SBUF), not via PE
    ones-matmuls into PSUM: frees 2 PSUM banks and avoids PE-FIFO stalls.
  - kv AllReduce chunked by head-quarter; phase 3 att starts as chunks land.
  - u/w weight loads + uT spill + y stores ride the scalar DMA queue so the
    sync queue keeps phase-critical traffic only; phase-1 issue order puts
    wq/wv ahead of the 8 MB xT load so the PE starts ~8us in, not 61us.
  - y is written bf16 (host upcasts); halves the output-DMA tail.
All matmuls bf16 (fp32 PSUM accumulate); elementwise/LN math fp32.
"""

import sys

if "/opt/trn_rl_repo" not in sys.path:
    sys.path.insert(0, "/opt/trn_rl_repo")

import numpy as np
import ml_dtypes

B, N, E = 4, 4096, 2048
H_DIM, HEADS, DH = 2048, 16, 128
N_CORES = 8
NL = N // N_CORES          # 512 tokens per (core, batch)
TL = B * NL                # 2048 local tokens per core
ET = E // 128              # 16 contraction tiles
JT = H_DIM // 128          # 16 hidden tiles
NJQ = 4                    # process hidden dim in 4 quarters of 512
TOK_B = NL // 128          # 4 token tiles per batch
LN_EPS = 1e-5

_BUILT = {}


def _build(flags, debug=False):
    import concourse.bacc as bacc
    import concourse.mybir as mybir
    import concourse.tile as tile

    has_bqv, has_bu, has_bo, has_affine = flags
    f32 = mybir.dt.float32
    bf16 = mybir.dt.bfloat16

    nc = bacc.Bacc("TRN2", target_bir_lowering=False, debug=False,
                   num_devices=N_CORES)

    t = {}
    t["xT"] = nc.dram_tensor("xT", [E, TL], bf16, kind="ExternalInput").ap()
    # weights arrive pre-rearranged from the host so every device load is
    # contiguous (strided 1KB-chunk DMAs run at ~34 GB/s vs 358 contiguous)
    t["wqk"] = nc.dram_tensor("wqk", [NJQ, 128, ET, 512], bf16,
                              kind="ExternalInput").ap()
    t["wv"] = nc.dram_tensor("wv", [NJQ, 128, ET, 512], bf16,
                             kind="ExternalInput").ap()
    t["wu"] = nc.dram_tensor("wu", [NJQ, 128, ET, 512], bf16,
                             kind="ExternalInput").ap()
    t["wo"] = nc.dram_tensor("wo", [128, JT, E], bf16,
                             kind="ExternalInput").ap()
    if has_bqv:
        t["bqk_r"] = nc.dram_tensor("bqk_r", [1, H_DIM], bf16,
                                    kind="ExternalInput").ap()
        t["bv_r"] = nc.dram_tensor("bv_r", [1, H_DIM], bf16,
                                   kind="ExternalInput").ap()
    if has_bo:
        t["bo_r"] = nc.dram_tensor("bo_r", [1, E], bf16,
                                   kind="ExternalInput").ap()
    if has_bu:
        t["bu_c"] = nc.dram_tensor("bu_c", [128, JT], f32,
                                   kind="ExternalInput").ap()
    if has_affine:
        t["g_c"] = nc.dram_tensor("g_c", [128, JT], f32,
                                  kind="ExternalInput").ap()
        t["b_c"] = nc.dram_tensor("b_c", [128, JT], f32,
                                  kind="ExternalInput").ap()
    t["y"] = nc.dram_tensor("y", [TL, E], bf16, kind="ExternalOutput").ap()

    dbg = None
    if debug:
        dbg = {
            "q": nc.dram_tensor("dbg_q", [TL, H_DIM], bf16,
                                kind="ExternalOutput").ap(),
            "uT": nc.dram_tensor("dbg_uT", [B * 128, JT * NL], bf16,
                                 kind="ExternalOutput").ap(),
            "kvout": nc.dram_tensor("dbg_kvout", [NJQ * B * 128, 4 * DH],
                                    f32, kind="ExternalOutput").ap(),
            "att": nc.dram_tensor("dbg_att", [B * 128, HEADS * NL], bf16,
                                  kind="ExternalOutput").ap(),
            "zT": nc.dram_tensor("dbg_zT", [B * 128, JT * NL], bf16,
                                 kind="ExternalOutput").ap(),
        }
    with tile.TileContext(nc) as tc:
        _body(nc, tc, tile, mybir, f32, bf16, t, flags, dbg)
    nc.compile()
    return nc


def _body(nc, tc, tile, mybir, f32, bf16, t, flags, dbg=None):
    Act = mybir.ActivationFunctionType
    Alu = mybir.AluOpType
    has_bqv, has_bu, has_bo, has_affine = flags

    with (
        tc.tile_pool(name="consts", bufs=1) as consts,
        tc.tile_pool(name="p3small", bufs=1) as p3small,
        tc.tile_pool(name="dram", bufs=1, space="DRAM") as dram,
    ):
        ones_col = consts.tile([128, 1], bf16)
        nc.vector.memset(ones_col, 1.0)
        eps_sb = consts.tile([1, 1], f32)
        nc.vector.memset(eps_sb, LN_EPS)
        utoken = consts.tile([1, 1], f32)
        nc.vector.memset(utoken, 0.0)
        if has_bqv or has_bo:
            ones_bf = consts.tile([1, 128], bf16)
            nc.vector.memset(ones_bf, 1.0)
        if has_bqv:
            bqk_sb = consts.tile([1, H_DIM], bf16)
            nc.sync.dma_start(bqk_sb[:], t["bqk_r"][:])
            bv_sb = consts.tile([1, H_DIM], bf16)
            nc.sync.dma_start(bv_sb[:], t["bv_r"][:])
        if has_bo:
            bo_sb = consts.tile([1, E], bf16)
            nc.sync.dma_start(bo_sb[:], t["bo_r"][:])
        if has_bu:
            bu_sb = consts.tile([128, JT], f32)
            nc.sync.dma_start(bu_sb[:], t["bu_c"][:])
        if has_affine:
            g_sb = consts.tile([128, JT], f32)
            nc.sync.dma_start(g_sb[:], t["g_c"][:])
            b_sb = consts.tile([128, JT], f32)
            nc.sync.dma_start(b_sb[:], t["b_c"][:])

        q_dram = dram.tile([TL, H_DIM], bf16)     # token-major q spill
        uT_dram = dram.tile([128, B, JT, NL], bf16)
        # kv collective chunks: one per head-quarter, laid out
        # [B, 128, 4, DH] so the kv stores/loads are direct 2KB-run DMAs
        cc_in = []
        cc_out = []
        for jq in range(NJQ):
            cc_in_t = dram.tile([B, 128, 4, DH], f32, name=f"cc_in{jq}")
            cc_in.append(cc_in_t)
            cc_out_t = dram.tile([B, 128, 4, DH], f32,
                                 addr_space="Shared", name=f"cc_out{jq}")
            cc_out.append(cc_out_t)

        # small early pools: their SBUF is disjoint from xt/w1, so the
        # phase-3 qb/kv loads can prefetch during phase 2 (no WAR on xt)
        kvf_ctx = tc.tile_pool(name="kvf", bufs=1)
        kvf = kvf_ctx.__enter__()
        qb_ctx = tc.tile_pool(name="qbp", bufs=1)
        qbp = qb_ctx.__enter__()

        with tc.tile_pool(name="xt_pool", bufs=1) as xt_pool:
            xt = xt_pool.tile([128, ET, TL], bf16)   # 8 MB resident ph1-2

            # w2 spans phases 1+2 with SBUF addresses disjoint from w1/st1,
            # so the wu prefetch overlaps phase 1 (no phase-boundary stall)
            w2_ctx = tc.tile_pool(name="w2", bufs=1)
            w2 = w2_ctx.__enter__()

            # ---------------- phase 1: q/v projections + partial kv --------
            with (
                tc.tile_pool(name="w1", bufs=1) as w1,
                tc.tile_pool(name="st1", bufs=1) as st1,
                tc.tile_pool(name="ps_proj", bufs=1, space="PSUM") as psp,
                tc.tile_pool(name="ps_kv", bufs=1, space="PSUM") as pskv,
            ):
                # phase-1 weights first in the DMA queue so the PE can start
                # ~8us in instead of waiting behind the 8 MB xT load
                wq_sb = {}
                wv_sb = {}
                wq_sb[0] = w1.tile([128, ET, 512], bf16, tag="wq", bufs=2,
                                   name="wq0")
                nc.sync.dma_start(wq_sb[0][:], t["wqk"][0])
                wv_sb[0] = w1.tile([128, ET, 512], bf16, tag="wv", bufs=2,
                                   name="wv0")
                nc.scalar.dma_start(wv_sb[0][:], t["wv"][0])

                for tt in range(ET):
                    nc.sync.dma_start(
                        xt[:, tt],
                        t["xT"].rearrange("(t p) n -> t p n", p=128)[tt])

                wu_pre = []
                for jqu in range(2):
                    wu_q = w2.tile([128, ET, 512], bf16, tag="wu", bufs=2)
                    nc.scalar.dma_start(wu_q[:], t["wu"][jqu])
                    wu_pre.append(wu_q)

                for jq in range(NJQ):
                    if jq > 0:
                        wq_sb[jq] = w1.tile([128, ET, 512], bf16, tag="wq",
                                            bufs=2, name=f"wq{jq}")
                        nc.sync.dma_start(wq_sb[jq][:], t["wqk"][jq])
                        wv_sb[jq] = w1.tile([128, ET, 512], bf16, tag="wv",
                                            bufs=2, name=f"wv{jq}")
                        nc.sync.dma_start(wv_sb[jq][:], t["wv"][jq])
                    for b in range(B):
                        q_tiles, v_tiles = [], []
                        for tk in range(TOK_B):
                            tok0 = b * NL + tk * 128
                            q_ps = psp.tile([128, 512], f32, tag="qps", bufs=2)
                            v_ps = psp.tile([128, 512], f32, tag="vps", bufs=2)
                            for tt in range(ET):
                                lhs = xt[:, tt, tok0:tok0 + 128]
                                nc.tensor.matmul(q_ps[:], lhs,
                                                 wq_sb[jq][:, tt],
                                                 start=(tt == 0), stop=False)
                                nc.tensor.matmul(
                                    v_ps[:], lhs, wv_sb[jq][:, tt],
                                    start=(tt == 0),
                                    stop=(not has_bqv and tt == ET - 1))
                            if has_bqv:
                                nc.tensor.matmul(
                                    q_ps[:], ones_bf[:],
                                    bqk_sb[:, jq * 512:(jq + 1) * 512],
                                    start=False, stop=True)
                                nc.tensor.matmul(
                                    v_ps[:], ones_bf[:],
                                    bv_sb[:, jq * 512:(jq + 1) * 512],
                                    start=False, stop=True)

                            # elu(q) = (max(q,0) - 1) + exp(min(q, 0))
                            tmin = st1.tile([128, 512], f32, tag="tmin",
                                            bufs=2)
                            nc.vector.tensor_scalar_min(tmin[:], q_ps[:], 0.0)
                            texp = st1.tile([128, 512], f32, tag="texp",
                                            bufs=1)
                            nc.scalar.activation(texp[:], tmin[:], Act.Exp)
                            trelu = st1.tile([128, 512], f32, tag="trelu",
                                             bufs=1)
                            nc.vector.tensor_scalar(trelu[:], q_ps[:], 0.0,
                                                    -1.0, Alu.max, Alu.add)
                            q_bf = st1.tile([128, 512], bf16, tag="qbf",
                                            bufs=4)
                            nc.vector.tensor_add(q_bf[:], trelu[:], texp[:])
                            # silu(v) = 0.5*v*(1+tanh(v/2)) — keeps the exp
                            # table set loaded (tanh lives in it; Silu not)
                            vth = st1.tile([128, 512], f32, tag="vth", bufs=1)
                            nc.scalar.activation(vth[:], v_ps[:], Act.Tanh,
                                                 scale=0.5)
                            vsg = st1.tile([128, 512], f32, tag="vsg", bufs=1)
                            nc.vector.tensor_scalar(vsg[:], vth[:], 0.5, 0.5,
                                                    Alu.mult, Alu.add)
                            v_bf = st1.tile([128, 512], bf16, tag="vbf",
                                            bufs=4)
                            nc.vector.tensor_mul(v_bf[:], v_ps[:], vsg[:])
                            q_tiles.append(q_bf)
                            v_tiles.append(v_bf)

                            # spill q token-major; phase 3 reloads it through
                            # the XBAR transposing DMA
                            nc.sync.dma_start(
                                q_dram[tok0:tok0 + 128,
                                       jq * 512:(jq + 1) * 512],
                                q_bf[:])

                        # per-head contiguous kv accumulation: each head owns
                        # a whole PSUM bank (start=True clears the full bank,
                        # so accumulation groups must not share banks)
                        kv_sb = st1.tile([128, 4, DH], f32, tag="kvsb",
                                         bufs=1)
                        for h in range(4):
                            kv_ps = pskv.tile([128, DH], f32, tag="kv",
                                              bufs=2)
                            for tk in range(TOK_B):
                                nc.tensor.matmul(
                                    kv_ps[:],
                                    q_tiles[tk][:, h * 128:(h + 1) * 128],
                                    v_tiles[tk][:, h * 128:(h + 1) * 128],
                                    start=(tk == 0), stop=(tk == TOK_B - 1))
                            nc.vector.tensor_copy(kv_sb[:, h], kv_ps[:])
                        nc.sync.dma_start(cc_in[jq][b], kv_sb[:])

                    # AllReduce this head-quarter as soon as its kv is out;
                    # chunks overlap the phase-1 tail + the u-projection
                    nc.gpsimd.collective_compute(
                        "AllReduce", mybir.AluOpType.add,
                        replica_groups=[list(range(N_CORES))],
                        ins=[cc_in[jq].opt()], outs=[cc_out[jq].opt()])

            # ---------------- phase 2: uT projection (overlaps AR) ---------
            u_st_last = [None]
            with (
                tc.tile_pool(name="u2", bufs=1) as u2,
                tc.tile_pool(name="ps_u", bufs=1, space="PSUM") as psu,
            ):
                for jqu in range(NJQ):
                    if jqu < 2:
                        wu_q = wu_pre[jqu]
                    else:
                        wu_q = w2.tile([128, ET, 512], bf16, tag="wu", bufs=2)
                        nc.scalar.dma_start(wu_q[:], t["wu"][jqu])
                    u_st = u2.tile([128, 4, TL], bf16, tag="ust", bufs=2)
                    for jl in range(4):
                        jt = jqu * 4 + jl
                        ubias = bu_sb[:, jt:jt + 1] if has_bu else 0.0
                        for ch in range(2):
                            u_ps = psu.tile([128, 2, 512], f32, tag="ups",
                                            bufs=2)
                            for tt in range(ET):
                                for c2 in range(2):
                                    c = ch * 2 + c2
                                    nc.tensor.matmul(
                                        u_ps[:, c2],
                                        wu_q[:, tt, jl * 128:(jl + 1) * 128],
                                        xt[:, tt, c * 512:(c + 1) * 512],
                                        start=(tt == 0), stop=(tt == ET - 1))
                            # silu via tanh keeps the exp table set loaded
                            # for the whole kernel (one switch total)
                            uth = u2.tile([128, 2, 512], f32, tag="uth",
                                          bufs=2)
                            nc.scalar.activation(uth[:], u_ps[:], Act.Tanh,
                                                 scale=0.5, bias=ubias)
                            usg = u2.tile([128, 2, 512], f32, tag="usg",
                                          bufs=2)
                            nc.vector.tensor_scalar(usg[:], uth[:], 0.5, 0.5,
                                                    Alu.mult, Alu.add)
                            nc.vector.tensor_mul(
                                u_st[:, jl, ch * 1024:(ch + 1) * 1024]
                                .rearrange("p (a n) -> p a n", a=2),
                                u_ps[:], usg[:])
                    u_st_last[0] = u_st
                    # contiguous spills per (quarter, batch): 4KB runs.
                    # Same ring (sync) as the phase-3 uT loads: ring FIFO
                    # order guarantees load-after-spill without relying on
                    # cross-ring DRAM dependency tracking.
                    for sb in range(B):
                        nc.scalar.dma_start(
                            uT_dram[:, sb, jqu * 4:(jqu + 1) * 4, :],
                            u_st[:, :, sb * NL:(sb + 1) * NL])
                # token carrying "all u activations done" for the chain dep
                nc.vector.tensor_copy(utoken[:], u_st_last[0][0:1, 0, 0:1])
            w2_ctx.__exit__(None, None, None)

        if dbg is not None:
            nc.sync.dma_start(dbg["q"][:], q_dram[:])
            nc.sync.dma_start(
                dbg["uT"].rearrange("(b p) (jt n) -> p b jt n", p=128, jt=JT),
                uT_dram[:])
            for jq in range(NJQ):
                nc.sync.dma_start(
                    dbg["kvout"][jq * B * 128:(jq + 1) * B * 128, :]
                    .rearrange("(b p) (h e) -> b p h e", b=B, h=4),
                    cc_out[jq][:])

        # ------------- phase 3+4: attention, layernorm, output proj --------
        with (
            tc.tile_pool(name="wo_pool", bufs=1) as wo_pool,
            tc.tile_pool(name="st3", bufs=1) as st3,
            tc.tile_pool(name="utp", bufs=1) as utp,
            tc.tile_pool(name="ps_att", bufs=1, space="PSUM") as psa,
            tc.tile_pool(name="ps_sm", bufs=1, space="PSUM") as pssm,
            tc.tile_pool(name="ps_y", bufs=1, space="PSUM") as psy,
        ):
            wo_sb = wo_pool.tile([128, JT, E], bf16)     # 8 MB resident
            for wq4 in range(4):
                weng = nc.scalar if wq4 % 2 == 0 else nc.sync
                weng.dma_start(wo_sb[:, wq4 * 4:(wq4 + 1) * 4],
                               t["wo"][:, wq4 * 4:(wq4 + 1) * 4])

            att_tiles = {}
            sq_tiles = {}
            stat_tiles = {}

            def att_block(b):
                kv_bf = kvf.tile([128, HEADS, DH], bf16, tag="kvbf", bufs=1)
                for jq in range(NJQ):
                    nc.gpsimd.dma_start(
                        kv_bf[:, jq * 4:(jq + 1) * 4], cc_out[jq][b])
                att = st3.tile([128, HEADS, NL], bf16, tag="att", bufs=2)
                att_tiles[b] = att
                sq = st3.tile([128, HEADS, NL], bf16, tag="sq", bufs=1)
                sq_tiles[b] = sq
                for jq in range(NJQ):
                    # one XBAR transpose per head-quarter (not per head):
                    # 16 loads of 512KB pipeline ahead of the att matmuls
                    qb = qbp.tile([128, 4, NL], bf16, tag="qb", bufs=4)
                    nc.sync.dma_start_transpose(
                        qb[:],
                        q_dram[b * NL:(b + 1) * NL,
                               jq * 512:(jq + 1) * 512])
                    for hl in range(4):
                        h = jq * 4 + hl
                        att_ps = psa.tile([128, NL], f32, tag="attps", bufs=3)
                        nc.tensor.matmul(att_ps[:], kv_bf[:, h], qb[:, hl],
                                         start=True, stop=True)
                        nc.vector.tensor_copy(att[:, h], att_ps[:])
                        nc.scalar.activation(sq[:, h], att[:, h], Act.Square)
                if dbg is not None:
                    nc.sync.dma_start(dbg["att"][b * 128:(b + 1) * 128, :],
                                      att[:])

            def stats_block(b):
                att = att_tiles[b]
                sq = sq_tiles.pop(b)
                # LN stats over channels via ones-matmuls (bf16 operands)
                sum_ps = pssm.tile([1, NL], f32, tag="sum", bufs=1)
                ssq_ps = pssm.tile([1, NL], f32, tag="ssq", bufs=1)
                for h in range(HEADS):
                    nc.tensor.matmul(sum_ps[:], ones_col[:], att[:, h],
                                     start=(h == 0), stop=(h == HEADS - 1))
                for h in range(HEADS):
                    nc.tensor.matmul(ssq_ps[:], ones_col[:], sq[:, h],
                                     start=(h == 0), stop=(h == HEADS - 1))
                stat_tiles[b] = (sum_ps, ssq_ps)

            def chain_block(b):
                # small [1,NL] tiles cost full per-partition column space, so
                # the chain recycles three tags by value lifetime
                sum_ps, ssq_ps = stat_tiles.pop(b)
                mean = st3.tile([1, NL], f32, tag="cA", bufs=1, name="mean")
                # bypass-op dep on utoken (written after the last u tile):
                # forces every phase-2 Tanh before the chain's Ln/Exp so the
                # ACT table set switches exactly once
                nc.vector.scalar_tensor_tensor(
                    mean[:], sum_ps[:], 1.0 / H_DIM,
                    utoken.broadcast_to([1, NL]), Alu.mult, Alu.bypass)
                m2 = st3.tile([1, NL], f32, tag="cB", bufs=1, name="m2")
                nc.scalar.activation(m2[:], mean[:], Act.Square)
                var = st3.tile([1, NL], f32, tag="cC", bufs=1, name="var")
                nc.vector.scalar_tensor_tensor(
                    var[:], ssq_ps[:], 1.0 / H_DIM, m2[:],
                    Alu.mult, Alu.subtract)
                lnv = st3.tile([1, NL], f32, tag="cB", bufs=1, name="lnv")
                nc.scalar.activation(lnv[:], var[:], Act.Ln, bias=eps_sb[:])
                rstd = st3.tile([1, NL], f32, tag="cC", bufs=1, name="rstd")
                nc.scalar.activation(rstd[:], lnv[:], Act.Exp, scale=-0.5)
                mr = st3.tile([1, NL], f32, tag="cB", bufs=1, name="mr")
                nc.vector.tensor_mul(mr[:], mean[:], rstd[:])
                rstd_b16 = st3.tile([1, NL], bf16, tag="c16a", bufs=1,
                                    name="rstd_b16")
                nc.vector.tensor_copy(rstd_b16[:], rstd[:])
                mr_b16 = st3.tile([1, NL], bf16, tag="c16b", bufs=1,
                                  name="mr_b16")
                nc.vector.tensor_copy(mr_b16[:], mr[:])
                rstd_bc = st3.tile([128, NL], bf16, tag="rstdbc", bufs=2)
                nc.gpsimd.partition_broadcast(rstd_bc[:], rstd_b16[:])
                mr_bc = st3.tile([128, NL], bf16, tag="mrbc", bufs=2)
                nc.gpsimd.partition_broadcast(mr_bc[:], mr_b16[:])
                return rstd_bc, mr_bc

            ut_loaded = {}

            def load_ut(b):
                uT_b = utp.tile([128, JT, NL], bf16, tag="utb", bufs=1)
                nc.sync.dma_start(uT_b[:], uT_dram[:, b])
                ut_loaded[b] = uT_b

            zT_tiles = {}

            def zT_block(b, bc):
                rstd_bc, mr_bc = bc
                att = att_tiles.pop(b)
                uT_b = ut_loaded.pop(b)
                zT = st3.tile([128, JT, NL], bf16, tag="zT", bufs=2)
                zT_tiles[b] = zT
                # (att*rstd - mean*rstd) [*g+b] * u — bf16, quarters split
                # across DVE and GpSimd so the zT latency halves
                for e8 in range(8):
                    eng = nc.vector
                    j0 = e8 * 2
                    s1 = st3.tile([128, 2, NL], bf16, tag=f"s1{e8 // 4}",
                                  bufs=1, name="s1")
                    eng.tensor_mul(
                        s1[:], att[:, j0:j0 + 2],
                        rstd_bc[:, None, :].broadcast_to([128, 2, NL]))
                    s2 = st3.tile([128, 2, NL], bf16, tag=f"s2{e8 // 4}",
                                  bufs=1, name="s2")
                    eng.tensor_sub(
                        s2[:], s1[:],
                        mr_bc[:, None, :].broadcast_to([128, 2, NL]))
                    if has_affine:
                        s3 = st3.tile([128, 2, NL], bf16, tag=f"s3{e8 // 4}",
                                      bufs=1, name="s3")
                        for jl in range(2):
                            jt = j0 + jl
                            eng.tensor_scalar(
                                s3[:, jl], s2[:, jl], g_sb[:, jt:jt + 1],
                                b_sb[:, jt:jt + 1], Alu.mult, Alu.add)
                    else:
                        s3 = s2
                    eng.tensor_mul(zT[:, j0:j0 + 2], s3[:],
                                   uT_b[:, j0:j0 + 2])
                if dbg is not None:
                    nc.sync.dma_start(dbg["zT"][b * 128:(b + 1) * 128, :],
                                      zT[:])

            def yproj_block(b):
                zT = zT_tiles.pop(b)
                # y = zT.T @ Wo + bo; e-quarter PSUM tiles (1 bank, bufs=3)
                # so the evacuation copies overlap the next quarter's matmuls
                for tsl in range(TOK_B):
                    ybuf = st3.tile([128, E], bf16, tag="ybuf", bufs=1)
                    for e4 in range(4):
                        e0 = e4 * 512
                        y_ps = psy.tile([128, 512], f32, tag="yps", bufs=3)
                        for ct in range(JT):
                            zslice = zT[:, ct, tsl * 128:(tsl + 1) * 128]
                            nc.tensor.matmul(
                                y_ps[:], zslice,
                                wo_sb[:, ct, e0:e0 + 512],
                                start=(ct == 0),
                                stop=(not has_bo and ct == JT - 1))
                        if has_bo:
                            nc.tensor.matmul(
                                y_ps[:], ones_bf[:],
                                bo_sb[:, e0:e0 + 512],
                                start=False, stop=True)
                        nc.scalar.copy(ybuf[:, e0:e0 + 512], y_ps[:])
                    nc.sync.dma_start(
                        t["y"][b * NL + tsl * 128: b * NL + (tsl + 1) * 128,
                               :],
                        ybuf[:])

            # software-pipelined schedule: PE stream is
            #   att0 att1 stats0 att2 stats1 att3 stats2 yproj0 stats3 ...
            # while each batch's LN chain (ACT/DVE/GpSimd) resolves under the
            # next batches' matmuls.
            # emission order doubles as queue priority: qb/kv/uT loads
            # first (so they prefetch during phase 2), wo quarters late,
            # each batch's LN chain resolving under the next batches' MMs
            load_ut(0)
            att_block(0)
            load_ut(1)
            att_block(1)
            stats_block(0)
            bc0 = chain_block(0)
            zT_block(0, bc0)
            att_block(2)
            load_ut(2)
            stats_block(1)
            bc1 = chain_block(1)
            zT_block(1, bc1)
            att_block(3)
            load_ut(3)
            stats_block(2)
            bc2 = chain_block(2)
            yproj_block(0)
            zT_block(2, bc2)
            stats_block(3)
            bc3 = chain_block(3)
            yproj_block(1)
            zT_block(3, bc3)
            yproj_block(2)
            yproj_block(3)

        qb_ctx.__exit__(None, None, None)
        kvf_ctx.__exit__(None, None, None)



def _get_nc(flags, debug=False):
    key = (flags, debug)
    if key not in _BUILT:
        _BUILT[key] = _build(flags, debug)
    return _BUILT[key]


def make_in_maps(x, Wqk, bqk, Wv, bv, Wu, bu, Wo, bo, ln_g, ln_b):
    bf16 = ml_dtypes.bfloat16
    f32 = np.float32
    x = np.asarray(x)
    flags = (
        bool(np.any(bqk) or np.any(bv)),
        bool(np.any(bu)),
        bool(np.any(bo)),
        bool(np.any(np.asarray(ln_g) != 1.0) or np.any(ln_b)),
    )
    def _wr(w):  # [E, H_DIM] -> [NJQ, 128, ET, 512] (jq, p, tt, j)
        return np.ascontiguousarray(
            np.asarray(w, f32).astype(bf16)
            .reshape(ET, 128, NJQ, 512).transpose(2, 1, 0, 3))

    shared = {
        "wqk": _wr(Wqk),
        "wv": _wr(Wv),
        "wu": _wr(Wu),
        "wo": np.ascontiguousarray(
            np.asarray(Wo, f32).astype(bf16)
            .reshape(JT, 128, E).transpose(1, 0, 2)),
    }
    if flags[0]:
        shared["bqk_r"] = np.asarray(bqk, f32).astype(bf16).reshape(1, H_DIM)
        shared["bv_r"] = np.asarray(bv, f32).astype(bf16).reshape(1, H_DIM)
    if flags[1]:
        shared["bu_c"] = np.ascontiguousarray(
            np.asarray(bu, f32).reshape(JT, 128).T)
    if flags[2]:
        shared["bo_r"] = np.asarray(bo, f32).astype(bf16).reshape(1, E)
    if flags[3]:
        shared["g_c"] = np.ascontiguousarray(
            np.asarray(ln_g, f32).reshape(JT, 128).T)
        shared["b_c"] = np.ascontiguousarray(
            np.asarray(ln_b, f32).reshape(JT, 128).T)
    in_maps = []
    for c in range(N_CORES):
        xc = np.ascontiguousarray(
            x[:, c * NL:(c + 1) * NL, :].reshape(TL, E).T).astype(bf16)
        in_maps.append({"xT": xc, **shared})
    return flags, in_maps


def kernel(x, Wqk, bqk, Wv, bv, Wu, bu, Wo, bo, ln_g, ln_b, **_unused):
    from concourse.bass_utils import run_bass_kernel_spmd

    flags, in_maps = make_in_maps(x, Wqk, bqk, Wv, bv, Wu, bu, Wo, bo,
                                  ln_g, ln_b)
    nc = _get_nc(flags)
    res = run_bass_kernel_spmd(nc, in_maps, core_ids=list(range(N_CORES)))

    y = np.empty((B, N, E), np.float32)
    for c in range(N_CORES):
        y[:, c * NL:(c + 1) * NL, :] = (
            res.results[c]["y"].astype(np.float32).reshape(B, NL, E))
    return y
